# revision 1
# baseline (speedup 1.0000x reference)
"""Trainium2 Bass kernel for nn_Encoder_90469191122997 (gnn_message_passing).

Data-parallel over batch B=8: core b owns batch b end-to-end.
Per core (x_b = x[:, b] as [T*C, HW] = [1152, 12544] fp32):
  pass 1: stream x_b, 2x2 box-downsample on DVE (masks are 56x56 nearest-
          upsampled, so pooling contracts exactly at 56x56), PE-transpose the
          downsampled tiles and matmul against transposed masks -> node
          features [18, 192] (and their transpose) accumulated in PSUM.
  GCN:    18x18 softmax adjacency + two linears + message passing, all
          on-chip via small PE matmuls / DVE / ACT.
  pass 2: re-stream x_b, PE matmul (outg x masks) -> residual at 56x56 in
          PSUM, DVE adds it (2x nearest-upsampled via step-0 broadcast APs)
          into x tiles, stream result to y.
Memory-bound: ~173 MB HBM traffic per core (2 reads + 1 write of x_b).
"""

import numpy as np

import concourse.bass as bass
import concourse.mybir as mybir
import concourse.tile as tile
from concourse.masks import make_identity
from concourse.vector_clock import ScopedClock

T, B, C, H, W = 6, 8, 192, 112, 112
K = 3
H0, W0 = 56, 56
HW = H * W            # 12544
HW0 = H0 * W0         # 3136
N = T * K             # 18
CH = 96               # c half
NJ = 25               # ceil(3136/128) transpose chunks per (t, ch)
NR = 7                # residual hw0 chunks of 448 per row-chunk
RW = 448              # residual chunk width at 56-res (8 rows of 56)
NCH = T * C // 128    # 9 row-chunks of 128 (t,c) rows each


def _spans(r):
    """(t, lo, hi, clo): rows [lo,hi) of chunk r belong to t, starting at
    channel clo.  Chunk boundaries hit t-edges only at offsets 0/64."""
    out = []
    for t in range(T):
        lo = max(128 * r, C * t)
        hi = min(128 * r + 128, C * (t + 1))
        if lo < hi:
            out.append((t, lo - 128 * r, hi - 128 * r, lo - C * t))
    return out


_LAST_CHUNK = {t: (C * (t + 1) - 1) // 128 for t in range(T)}

_MAX_WAITS = 1


def _split_multi_waits(nc):
    """This container's walrus rejects >1 sem wait per instruction ("Too many
    sync wait commands").  Move extra waits onto same-engine NoOps inserted
    immediately before the instruction (per-engine program order preserved)."""
    for bb in nc.main_func.blocks:
        insts = list(bb.instructions)
        if not any(
            i.sync_info and i.sync_info.on_wait
            and len(i.sync_info.on_wait) > _MAX_WAITS
            for i in insts
        ):
            continue
        new = []
        for inst in insts:
            si = inst.sync_info
            if si and si.on_wait and len(si.on_wait) > _MAX_WAITS:
                extra = list(si.on_wait[_MAX_WAITS:])
                del si.on_wait[_MAX_WAITS:]
                while extra:
                    chunk, extra = extra[:_MAX_WAITS], extra[_MAX_WAITS:]
                    nop = mybir.InstNoOp(
                        name=nc.get_next_instruction_name(),
                        engine=inst.engine,
                        bass_nofuse=True,
                        sync_info=mybir.SyncInfo(on_wait=chunk, on_update=[]),
                    )
                    nc.register_instruction(nop, overwrite=True)
                    new.append(nop)
            new.append(inst)
        bb.instructions = new


_orig_drain_and_barrier = tile.TileContext._drain_and_barrier


def _patched_drain_and_barrier(self, tick_clock, wait_clock):
    _orig_drain_and_barrier(self, tick_clock, wait_clock)
    _split_multi_waits(self.nc)


tile.TileContext._drain_and_barrier = _patched_drain_and_barrier

F32 = mybir.dt.float32


KNOBS = dict(x2_bufs=2, tr_bufs=3, x2T_bufs=6, res_bufs=4,
             skip_pool=False, skip_res=False, skip_store=False,
             store_eng='scalar')


def build_nc(reps: int = 1, x_bufs: int = 3, step_a_gpsimd: bool = False) -> bass.Bass:
    nc = bass.Bass()
    x = nc.dram_tensor("x", [T * C, HW], F32, kind="ExternalInput")
    m56 = nc.dram_tensor("m56", [N, HW0], F32, kind="ExternalInput")
    mTp = nc.dram_tensor("mTp", [128, T * NJ * K], F32, kind="ExternalInput")
    wembT = nc.dram_tensor("wembT", [C, C], F32, kind="ExternalInput")
    wgcn = nc.dram_tensor("wgcn", [C, C], F32, kind="ExternalInput")
    bb = nc.dram_tensor("bb", [N, C], F32, kind="ExternalInput")
    y = nc.dram_tensor("y", [T * C, HW], F32, kind="ExternalOutput")

    with tile.TileContext(nc) as tc:
        with (
            tc.tile_pool(name="persist", bufs=1) as pp,
            tc.tile_pool(name="xpool", bufs=x_bufs) as xpool,
            tc.tile_pool(name="x2Tpool", bufs=KNOBS["x2T_bufs"]) as x2Tpool,
            tc.tile_pool(name="smallsb", bufs=2) as ssb,
        ):
            ident = pp.tile([128, 128], F32)
            make_identity(nc, ident)
            mTp_sb = pp.tile([128, T * NJ * K], F32)
            nc.sync.dma_start(mTp_sb[:], mTp[:])
            wemb_h = []
            wgcn_h = []
            for hh in range(2):
                wt = pp.tile([CH, C], F32, tag=f"wemb{hh}")
                nc.sync.dma_start(wt[:], wembT[hh * CH:(hh + 1) * CH, :])
                wemb_h.append(wt)
                gt = pp.tile([CH, C], F32, tag=f"wgcn{hh}")
                nc.sync.dma_start(gt[:], wgcn[hh * CH:(hh + 1) * CH, :])
                wgcn_h.append(gt)
            bb_sb = pp.tile([N, C], F32)
            nc.sync.dma_start(bb_sb[:], bb[:])

            for rep in range(reps):
                nodeT_h = [
                    pp.tile([CH, N], F32, tag=f"nodeT{hh}", name=f"nodeT{hh}") for hh in range(2)
                ]
                outg_t = [
                    pp.tile([N, C], F32, tag=f"outg_t{t}", name=f"outg_t{t}") for t in range(T)
                ]
                if KNOBS["skip_pool"]:  # timing experiment only
                    for hh in range(2):
                        nc.any.memset(nodeT_h[hh][:], 0.0)

                # ---------------- pass 1: pooling ----------------
                # Stream x as 9 chunks of 128 rows (full DMA-port width).
                # A chunk's rows span 1-2 t's; per-t work slices columns of
                # the transposed tile, so everything stays at partition base 0.
                with (
                    tc.tile_pool(name="x2pool", bufs=KNOBS["x2_bufs"]) as x2pool,
                    tc.tile_pool(name="trps", bufs=KNOBS["tr_bufs"], space="PSUM") as trps,
                    tc.tile_pool(name="featps", bufs=3, space="PSUM") as fps,
                    tc.tile_pool(name="ntps", bufs=2, space="PSUM") as ntps,
                ):
                    feat_ps = {}
                    for r in range(NCH):
                        xt = xpool.tile([128, HW], F32, tag="xt")
                        nc.sync.dma_start(xt[:], x[128 * r:128 * (r + 1), :])
                        if KNOBS["skip_pool"]:
                            continue
                        x3 = xt.rearrange("p (h w) -> p h w", w=W)
                        x2 = x2pool.tile([128, HW0], F32, tag="x2")
                        x23 = x2.rearrange("p (h w) -> p h w", w=W0)
                        # 2x2 box sum: three accumulating strided adds per
                        # h-half (no intermediate tile needed)
                        for hf in range(2):
                            out = x23[:, hf * (H0 // 2):(hf + 1) * (H0 // 2)]
                            a = x3[:, hf * (H // 2):(hf + 1) * (H // 2)]
                            nc.vector.tensor_add(out, a[:, ::2, ::2], a[:, ::2, 1::2])
                            nc.vector.tensor_add(out, out, a[:, 1::2, ::2])
                            nc.vector.tensor_add(out, out, a[:, 1::2, 1::2])
                        for (t, lo, hi, clo) in _spans(r):
                            if t not in feat_ps:
                                feat_ps[t] = fps.tile(
                                    [K, C], F32, tag="feat_ps", name=f"featps{t}"
                                )
                        for j in range(NJ):
                            wj = min(128, HW0 - j * 128)
                            tr = trps.tile([128, 128], F32, tag="tr")
                            nc.tensor.transpose(
                                tr[:wj, :],
                                x2[:, j * 128:j * 128 + wj],
                                ident[:, :],
                            )
                            x2T = x2Tpool.tile([128, 128], F32, tag="x2T")
                            if wj < 128:
                                nc.any.memset(x2T[wj:, :], 0.0)
                            nc.any.tensor_copy(x2T[:wj, :], tr[:wj, :])
                            for (t, lo, hi, clo) in _spans(r):
                                col = (t * NJ + j) * K
                                nc.tensor.matmul(
                                    feat_ps[t][:, clo:clo + (hi - lo)],
                                    mTp_sb[:, col:col + K],
                                    x2T[:, lo:hi],
                                    start=(j == 0),
                                    stop=(j == NJ - 1),
                                    skip_group_check=True,
                                )
                        for (t, lo, hi, clo) in _spans(r):
                            if _LAST_CHUNK[t] != r:
                                continue
                            feat_sb = ssb.tile([K, C], F32, tag="feat_sb")
                            nc.scalar.mul(feat_sb[:], feat_ps.pop(t)[:], 1.0 / HW)
                            for hh in range(2):
                                ntr = ntps.tile([CH, K], F32, tag="ntr")
                                nc.tensor.transpose(
                                    ntr[:],
                                    feat_sb[:, hh * CH:(hh + 1) * CH],
                                    ident[:K, :K],
                                )
                                nc.any.tensor_copy(
                                    nodeT_h[hh][:, K * t:K * (t + 1)], ntr[:]
                                )

                # ---------------- GCN on [18, 192] ----------------
                with tc.tile_pool(name="gcnps", bufs=1, space="PSUM") as gps:
                    adjL = gps.tile([N, N], F32, tag="adjL")
                    for hh in range(2):
                        nc.tensor.matmul(
                            adjL[:], nodeT_h[hh][:], nodeT_h[hh][:],
                            start=(hh == 0), stop=(hh == 1),
                        )
                    mx = ssb.tile([N, 1], F32, tag="mx")
                    nc.vector.reduce_max(mx[:], adjL[:], axis=mybir.AxisListType.X)
                    nmx = ssb.tile([N, 1], F32, tag="nmx")
                    nc.vector.tensor_scalar_mul(nmx[:], mx[:], -1.0)
                    e_sb = ssb.tile([N, N], F32, tag="e_sb")
                    nc.scalar.activation(
                        e_sb[:], adjL[:], mybir.ActivationFunctionType.Exp,
                        bias=nmx[:], scale=1.0,
                    )
                    s_ = ssb.tile([N, 1], F32, tag="s_")
                    nc.vector.reduce_sum(s_[:], e_sb[:], axis=mybir.AxisListType.X)
                    r_ = ssb.tile([N, 1], F32, tag="r_")
                    nc.vector.reciprocal(r_[:], s_[:])
                    adj_sb = ssb.tile([N, N], F32, tag="adj_sb")
                    nc.vector.tensor_scalar_mul(adj_sb[:], e_sb[:], r_[:])

                    aaa_ps = gps.tile([N, C], F32, tag="aaa_ps")
                    for hh in range(2):
                        nc.tensor.matmul(
                            aaa_ps[:], nodeT_h[hh][:], wemb_h[hh][:],
                            start=(hh == 0), stop=(hh == 1),
                        )
                    aaa_sb = ssb.tile([N, C], F32, tag="aaa_sb")
                    nc.scalar.copy(aaa_sb[:], aaa_ps[:])
                    aaaT_h = []
                    for hh in range(2):
                        aT_ps = gps.tile([CH, N], F32, tag="aT_ps")
                        nc.tensor.transpose(
                            aT_ps[:], aaa_sb[:, hh * CH:(hh + 1) * CH],
                            ident[:N, :N],
                        )
                        aT = ssb.tile([CH, N], F32, tag=f"aaaT{hh}")
                        nc.scalar.copy(aT[:], aT_ps[:])
                        aaaT_h.append(aT)
                    supp_ps = gps.tile([N, C], F32, tag="supp_ps")
                    for hh in range(2):
                        nc.tensor.matmul(
                            supp_ps[:], aaaT_h[hh][:], wgcn_h[hh][:],
                            start=(hh == 0), stop=(hh == 1),
                        )
                    supp_sb = ssb.tile([N, C], F32, tag="supp_sb")
                    nc.scalar.copy(supp_sb[:], supp_ps[:])
                    adjT_ps = gps.tile([N, N], F32, tag="adjT_ps")
                    nc.tensor.transpose(adjT_ps[:], adj_sb[:], ident[:N, :N])
                    adjT_sb = ssb.tile([N, N], F32, tag="adjT_sb")
                    nc.scalar.copy(adjT_sb[:], adjT_ps[:])
                    outg_ps = gps.tile([N, C], F32, tag="outg_ps")
                    nc.tensor.matmul(
                        outg_ps[:], adjT_sb[:], supp_sb[:], start=True, stop=True
                    )
                    outg_sb = ssb.tile([N, C], F32, tag="outg_sb")
                    nc.vector.tensor_add(outg_sb[:], outg_ps[:], bb_sb[:])
                    # zero-padded per-t copies so residual matmuls contract
                    # P=18 with partition base 0
                    for t in range(T):
                        nc.any.memset(outg_t[t][:], 0.0)
                        nc.sync.dma_start(
                            outg_t[t][K * t:K * (t + 1), :],
                            outg_sb[K * t:K * (t + 1), :],
                        )

                # ---------------- pass 2: residual ----------------
                # lhsT_r[r] is a block-"diagonal" [18, 128] tile whose column
                # range for each t-span holds outg rows 3t:3t+3 (zeros
                # elsewhere), so one matmul per (chunk, hw0-slice) produces
                # the residual for all 128 rows at partition base 0.
                with (
                    tc.tile_pool(name="m56pool", bufs=1) as mpool,
                    tc.tile_pool(name="resps", bufs=KNOBS["res_bufs"], space="PSUM") as rps,
                ):
                    m56_sb = mpool.tile([N, HW0], F32)
                    nc.sync.dma_start(m56_sb[:], m56[:])
                    lhsT_r = []
                    for r in range(NCH):
                        L = mpool.tile([N, 128], F32, name=f"lhsr{r}", tag=f"lhsr{r}")
                        for (t, lo, hi, clo) in _spans(r):
                            nc.any.tensor_copy(
                                L[:, lo:hi], outg_t[t][:, clo:clo + (hi - lo)]
                            )
                        lhsT_r.append(L)
                    for r in range(NCH):
                        xt2 = xpool.tile([128, HW], F32, tag="xt")
                        nc.sync.dma_start(xt2[:], x[128 * r:128 * (r + 1), :])
                        x5 = xt2.rearrange(
                            "p (h hh w ww) -> p h hh w ww",
                            h=H0, hh=2, w=W0, ww=2,
                        )
                        for j in range(NR if not KNOBS["skip_res"] else 0):
                            res = rps.tile([128, RW], F32, tag="res")
                            nc.tensor.matmul(
                                res[:],
                                lhsT_r[r][:],
                                m56_sb[:, j * RW:(j + 1) * RW],
                                start=True, stop=True,
                            )
                            # ISA caps APs at 3 free dims: split the h-repeat
                            # into two adds (even/odd rows)
                            r4 = res.rearrange("p (h w) -> p h w", w=W0)[
                                :, :, :, None
                            ].to_broadcast((128, 8, W0, 2))
                            for dh in range(2):
                                xs = x5[:, 8 * j:8 * (j + 1), dh]
                                nc.vector.tensor_add(xs, xs, r4)
                        st_eng = getattr(nc, KNOBS["store_eng"])
                        if not KNOBS["skip_store"]:
                            st_eng.dma_start(y[128 * r:128 * (r + 1), :], xt2[:])
                        elif r == 0:
                            nc.sync.dma_start(y[:1, :], xt2[:1, :])
    return nc


def _host_prep(x, gcn_masks, W_emb, W_gcn, b_gcn):
    x = np.asarray(x, dtype=np.float32)
    gcn_masks = np.asarray(gcn_masks)
    W_emb = np.asarray(W_emb, dtype=np.float32)
    W_gcn = np.asarray(W_gcn, dtype=np.float32)
    b_gcn = np.asarray(b_gcn, dtype=np.float32)
    wembT = np.ascontiguousarray(W_emb.T)
    bbv = np.ascontiguousarray(np.broadcast_to(b_gcn[None, :], (N, C)))
    in_maps = []
    for b in range(B):
        xb = np.ascontiguousarray(x[:, b]).reshape(T * C, HW)
        m = gcn_masks[b].reshape(T, K, HW0).astype(np.float32)
        m56v = np.ascontiguousarray(m.reshape(N, HW0))
        mp = np.zeros((T, K, NJ * 128), np.float32)
        mp[:, :, :HW0] = m
        mTpv = np.ascontiguousarray(
            mp.reshape(T, K, NJ, 128).transpose(3, 0, 2, 1).reshape(128, T * NJ * K)
        )
        in_maps.append({
            "x": xb, "m56": m56v, "mTp": mTpv,
            "wembT": wembT, "wgcn": np.ascontiguousarray(W_gcn), "bb": bbv,
        })
    return in_maps


_NC_CACHE = {}


def kernel(x, gcn_masks, W_emb, W_gcn, b_gcn):
    from concourse.bass_utils import run_bass_kernel_spmd

    in_maps = _host_prep(x, gcn_masks, W_emb, W_gcn, b_gcn)
    if "nc" not in _NC_CACHE:
        _NC_CACHE["nc"] = build_nc(reps=1)
    nc = _NC_CACHE["nc"]
    res = run_bass_kernel_spmd(nc, in_maps, list(range(B)))
    out = np.empty((T, B, C, H, W), np.float32)
    for b in range(B):
        out[:, b] = res.results[b]["y"].reshape(T, C, H, W)
    return out



# revision 5
# speedup vs baseline: 2.2068x; 2.2068x over previous
"""Trainium2 Bass kernel for nn_Encoder_90469191122997 (gnn_message_passing).

Data-parallel over batch B=8: core b owns batch b end-to-end.

v2: 16-bit traffic + SBUF residency.  x is sent to the device as bf16
(host cast), y is returned as bf16 (host upcast); the 2e-2 rel-err gate
leaves ~5x margin.  Per core x_b = [T*C, HW] bf16 = 28.9 MB; 7 of the 9
128-row chunks stay resident in SBUF between the pooling pass and the
residual pass, chunk 8 stays in the rotating buffer, and only chunk 7 is
re-read.  HBM traffic/core: 28.9 (read) + 3.2 (re-read) + 28.9 (write)
= 61 MB vs 173 MB for the fp32 two-pass baseline.

Pooling uses a 112-column grid (28 blocks x 112 = 3136, no tail).  Per
chunk, either:
  - DVE path: 2x2 box-sum on DVE (bf16) -> x2, then 28 PE transposes, or
  - PE path (KNOBS['pe_chunks']): 4 accumulating PE matmuls per block
    (x strided slice ^T @ identity) produce the box-summed transpose
    directly in fp32 PSUM, freeing DVE for the pass-2 residual adds.
Transposed tiles are copied PSUM->SBUF on ACT (KNOBS['copy_eng']) and
contracted against the pre-transposed masks -> node features [18, 192].
The 18-node GCN (softmax adjacency, two linears, message passing) runs
on-chip in fp32 PSUM with bf16 operands.  Pass 2 matmuls outg against
the 56x56 masks and DVE adds the 2x-nearest-upsampled residual into the
resident x tiles (broadcast APs), which are then DMA'd out as y.
"""

import numpy as np
import ml_dtypes

import concourse.bass as bass
import concourse.mybir as mybir
import concourse.tile as tile
from concourse.masks import make_identity

T, B, C, H, W = 6, 8, 192, 112, 112
K = 3
H0, W0 = 56, 56
HW = H * W            # 12544
HW0 = H0 * W0         # 3136
N = T * K             # 18
CH = 96               # c half
NJ = 28               # pooling blocks per chunk (112-col grid, no tail)
JW = 112              # pooling block width
NR = 7                # residual hw0 chunks of 448 per row-chunk
RW = 448              # residual chunk width at 56-res (8 rows of 56)
NCH = T * C // 128    # 9 row-chunks of 128 (t,c) rows each
NSTASH = 6            # chunks 0..5 resident in SBUF; 6,7 re-read; 8 in rot

BF = mybir.dt.bfloat16
F32 = mybir.dt.float32
BF_NP = ml_dtypes.bfloat16


def _spans(r):
    """(t, lo, hi, clo): rows [lo,hi) of chunk r belong to t, starting at
    channel clo.  Chunk boundaries hit t-edges only at offsets 0/64."""
    out = []
    for t in range(T):
        lo = max(128 * r, C * t)
        hi = min(128 * r + 128, C * (t + 1))
        if lo < hi:
            out.append((t, lo - 128 * r, hi - 128 * r, lo - C * t))
    return out


_LAST_CHUNK = {t: (C * (t + 1) - 1) // 128 for t in range(T)}

_MAX_WAITS = 1


def _split_multi_waits(nc):
    """This container's walrus rejects >1 sem wait per instruction ("Too many
    sync wait commands").  Move extra waits onto same-engine NoOps inserted
    immediately before the instruction (per-engine program order preserved)."""
    for bb in nc.main_func.blocks:
        insts = list(bb.instructions)
        if not any(
            i.sync_info and i.sync_info.on_wait
            and len(i.sync_info.on_wait) > _MAX_WAITS
            for i in insts
        ):
            continue
        new = []
        for inst in insts:
            si = inst.sync_info
            if si and si.on_wait and len(si.on_wait) > _MAX_WAITS:
                extra = list(si.on_wait[_MAX_WAITS:])
                del si.on_wait[_MAX_WAITS:]
                while extra:
                    chunk, extra = extra[:_MAX_WAITS], extra[_MAX_WAITS:]
                    nop = mybir.InstNoOp(
                        name=nc.get_next_instruction_name(),
                        engine=inst.engine,
                        bass_nofuse=True,
                        sync_info=mybir.SyncInfo(on_wait=chunk, on_update=[]),
                    )
                    nc.register_instruction(nop, overwrite=True)
                    new.append(nop)
            new.append(inst)
        bb.instructions = new


_orig_drain_and_barrier = tile.TileContext._drain_and_barrier


def _patched_drain_and_barrier(self, tick_clock, wait_clock):
    _orig_drain_and_barrier(self, tick_clock, wait_clock)
    _split_multi_waits(self.nc)


tile.TileContext._drain_and_barrier = _patched_drain_and_barrier


KNOBS = dict(
    pe_chunks=(),          # chunks whose box-sum+transpose runs on PE
    copy_eng='scalar',     # engine for PSUM->SBUF transpose-tile copies
    add_eng='vector',      # engine for pass-2 residual adds
    store_eng='scalar',    # engine issuing y store DMAs
    x2T_bufs=4, tr_bufs=2, res_bufs=4,
)


def build_nc(reps: int = 1) -> bass.Bass:
    nc = bass.Bass()
    x = nc.dram_tensor("x", [T * C, HW], BF, kind="ExternalInput")
    m56 = nc.dram_tensor("m56", [N, HW0], BF, kind="ExternalInput")
    mTp = nc.dram_tensor("mTp", [JW, T * NJ * K], BF, kind="ExternalInput")
    wembT = nc.dram_tensor("wembT", [C, C], BF, kind="ExternalInput")
    wgcn = nc.dram_tensor("wgcn", [C, C], BF, kind="ExternalInput")
    bb = nc.dram_tensor("bb", [N, C], F32, kind="ExternalInput")
    y = nc.dram_tensor("y", [T * C, HW], BF, kind="ExternalOutput")

    copy_eng = getattr(nc, KNOBS['copy_eng'])
    add_eng = getattr(nc, KNOBS['add_eng'])
    store_eng = getattr(nc, KNOBS['store_eng'])

    with tile.TileContext(nc) as tc:
        with (
            tc.tile_pool(name="persist", bufs=1) as pp,
            tc.tile_pool(name="x2Tpool", bufs=KNOBS['x2T_bufs']) as x2Tpool,
            tc.tile_pool(name="smallsb", bufs=2) as ssb,
        ):
            ident = pp.tile([128, 128], BF)
            make_identity(nc, ident)
            mTp_sb = pp.tile([JW, T * NJ * K], BF)
            nc.sync.dma_start(mTp_sb[:], mTp[:])
            m56_sb = pp.tile([N, HW0], BF)
            nc.sync.dma_start(m56_sb[:], m56[:])
            wemb_h = []
            wgcn_h = []
            for hh in range(2):
                wt = pp.tile([CH, C], BF, tag=f"wemb{hh}")
                nc.sync.dma_start(wt[:], wembT[hh * CH:(hh + 1) * CH, :])
                wemb_h.append(wt)
                gt = pp.tile([CH, C], BF, tag=f"wgcn{hh}")
                nc.sync.dma_start(gt[:], wgcn[hh * CH:(hh + 1) * CH, :])
                wgcn_h.append(gt)
            bb_sb = pp.tile([N, C], F32)
            nc.sync.dma_start(bb_sb[:], bb[:])

            # resident x chunks + rotating buffer + box-sum scratch
            st = [
                pp.tile([128, HW], BF, tag=f"stash{i}", name=f"stash{i}")
                for i in range(NSTASH)
            ]
            rot = pp.tile([128, HW], BF, tag="rot", name="rot")
            x2 = pp.tile([128, HW0], BF, tag="x2", name="x2")

            def chunk_buf(r):
                return st[r] if r < NSTASH else rot

            for rep in range(reps):
                nodeT_h = [
                    pp.tile([CH, N], BF, tag=f"nodeT{hh}", name=f"nodeT{hh}")
                    for hh in range(2)
                ]
                outgb = pp.tile([N, C], BF, tag="outgb", name="outgb")
                outg_t = [
                    pp.tile([N, C], BF, tag=f"outg_t{t}", name=f"outg_t{t}")
                    for t in range(T)
                ]
                lhsr = [
                    pp.tile([N, 128], BF, tag=f"lhsr{r}", name=f"lhsr{r}")
                    for r in range(NCH)
                ]

                # ---------------- pass 1: pooling ----------------
                with (
                    tc.tile_pool(name="trfps", bufs=KNOBS['tr_bufs'],
                                 space="PSUM") as trfps,
                    tc.tile_pool(name="trbps", bufs=KNOBS['tr_bufs'],
                                 space="PSUM") as trbps,
                    tc.tile_pool(name="featps", bufs=3, space="PSUM") as fps,
                    tc.tile_pool(name="ntps", bufs=1, space="PSUM") as ntps,
                ):
                    feat_ps = {}

                    def do_block(r, j, buf, x3):
                        """Produce x2T tile [112, 128] for block j of chunk r
                        and run its pooling matmuls."""
                        if r in KNOBS['pe_chunks']:
                            # 4 accumulating matmuls: out = slice^T (f32)
                            tr = trfps.tile([JW, 128], F32, tag="trf")
                            for q, (dh, dw) in enumerate(
                                ((0, 0), (0, 1), (1, 0), (1, 1))
                            ):
                                lhs = x3[:, 4 * j + dh:4 * j + dh + 3:2,
                                         dw::2]
                                nc.tensor.matmul(
                                    tr[:], lhs, ident[:],
                                    start=(q == 0), stop=(q == 3),
                                    skip_group_check=True,
                                )
                        else:
                            tr = trbps.tile([JW, 128], BF, tag="trb")
                            nc.tensor.transpose(
                                tr[:], x2[:, j * JW:(j + 1) * JW], ident[:]
                            )
                        x2T = x2Tpool.tile([JW, 128], BF, tag="x2T")
                        if hasattr(copy_eng, 'tensor_copy'):
                            copy_eng.tensor_copy(x2T[:], tr[:])
                        else:
                            copy_eng.copy(x2T[:], tr[:])
                        for (t, lo, hi, clo) in _spans(r):
                            col = (t * NJ + j) * K
                            nc.tensor.matmul(
                                feat_ps[t][:, clo:clo + (hi - lo)],
                                mTp_sb[:, col:col + K],
                                x2T[:, lo:hi],
                                start=(j == 0), stop=(j == NJ - 1),
                                skip_group_check=True,
                            )

                    for r in range(NCH):
                        buf = chunk_buf(r)
                        nc.sync.dma_start(buf[:], x[128 * r:128 * (r + 1), :])
                        x3 = buf.rearrange("p (h w) -> p h w", w=W)
                        for (t, lo, hi, clo) in _spans(r):
                            if t not in feat_ps:
                                feat_ps[t] = fps.tile(
                                    [K, C], F32, tag="feat_ps",
                                    name=f"featps{t}",
                                )
                        if r in KNOBS['pe_chunks']:
                            for j in range(NJ):
                                do_block(r, j, buf, x3)
                        else:
                            x23 = x2.rearrange("p (h w) -> p h w", w=W0)
                            # box-sum in two h-halves so PE transposes of the
                            # first half overlap DVE summing the second
                            for hf in range(2):
                                out = x23[:, hf * (H0 // 2):(hf + 1) * (H0 // 2)]
                                a = x3[:, hf * (H // 2):(hf + 1) * (H // 2)]
                                nc.vector.tensor_add(out, a[:, ::2, ::2],
                                                     a[:, ::2, 1::2])
                                nc.vector.tensor_add(out, out, a[:, 1::2, ::2])
                                nc.vector.tensor_add(out, out, a[:, 1::2, 1::2])
                                for j in range(hf * (NJ // 2),
                                               (hf + 1) * (NJ // 2)):
                                    do_block(r, j, buf, x3)
                        for (t, lo, hi, clo) in _spans(r):
                            if _LAST_CHUNK[t] != r:
                                continue
                            feat_sb = ssb.tile([K, C], BF, tag="feat_sb")
                            nc.scalar.mul(feat_sb[:], feat_ps.pop(t)[:],
                                          1.0 / HW)
                            for hh in range(2):
                                ntr = ntps.tile([CH, K], BF, tag="ntr")
                                nc.tensor.transpose(
                                    ntr[:],
                                    feat_sb[:, hh * CH:(hh + 1) * CH],
                                    ident[:K, :K],
                                )
                                nc.any.tensor_copy(
                                    nodeT_h[hh][:, K * t:K * (t + 1)], ntr[:]
                                )

                # ---------------- GCN on [18, 192] ----------------
                with tc.tile_pool(name="gcnps", bufs=1, space="PSUM") as gps:
                    adjL = gps.tile([N, N], F32, tag="adjL")
                    for hh in range(2):
                        nc.tensor.matmul(
                            adjL[:], nodeT_h[hh][:], nodeT_h[hh][:],
                            start=(hh == 0), stop=(hh == 1),
                        )
                    mx = ssb.tile([N, 1], F32, tag="mx")
                    nc.vector.reduce_max(mx[:], adjL[:], axis=mybir.AxisListType.X)
                    nmx = ssb.tile([N, 1], F32, tag="nmx")
                    nc.vector.tensor_scalar_mul(nmx[:], mx[:], -1.0)
                    e_sb = ssb.tile([N, N], F32, tag="e_sb")
                    nc.scalar.activation(
                        e_sb[:], adjL[:], mybir.ActivationFunctionType.Exp,
                        bias=nmx[:], scale=1.0,
                    )
                    s_ = ssb.tile([N, 1], F32, tag="s_")
                    nc.vector.reduce_sum(s_[:], e_sb[:], axis=mybir.AxisListType.X)
                    r_ = ssb.tile([N, 1], F32, tag="r_")
                    nc.vector.reciprocal(r_[:], s_[:])
                    adj_b = ssb.tile([N, N], BF, tag="adj_b")
                    nc.vector.tensor_scalar_mul(adj_b[:], e_sb[:], r_[:])

                    aaa_ps = gps.tile([N, C], F32, tag="aaa_ps")
                    for hh in range(2):
                        nc.tensor.matmul(
                            aaa_ps[:], nodeT_h[hh][:], wemb_h[hh][:],
                            start=(hh == 0), stop=(hh == 1),
                        )
                    aaa_b = ssb.tile([N, C], BF, tag="aaa_b")
                    nc.scalar.copy(aaa_b[:], aaa_ps[:])
                    aaaT_h = []
                    for hh in range(2):
                        aT_ps = gps.tile([CH, N], BF, tag="aT_ps")
                        nc.tensor.transpose(
                            aT_ps[:], aaa_b[:, hh * CH:(hh + 1) * CH],
                            ident[:N, :N],
                        )
                        aT = ssb.tile([CH, N], BF, tag=f"aaaT{hh}")
                        nc.scalar.copy(aT[:], aT_ps[:])
                        aaaT_h.append(aT)
                    supp_ps = gps.tile([N, C], F32, tag="supp_ps")
                    for hh in range(2):
                        nc.tensor.matmul(
                            supp_ps[:], aaaT_h[hh][:], wgcn_h[hh][:],
                            start=(hh == 0), stop=(hh == 1),
                        )
                    supp_b = ssb.tile([N, C], BF, tag="supp_b")
                    nc.scalar.copy(supp_b[:], supp_ps[:])
                    adjT_ps = gps.tile([N, N], BF, tag="adjT_ps")
                    nc.tensor.transpose(adjT_ps[:], adj_b[:], ident[:N, :N])
                    adjT_b = ssb.tile([N, N], BF, tag="adjT_b")
                    nc.scalar.copy(adjT_b[:], adjT_ps[:])
                    outg_ps = gps.tile([N, C], F32, tag="outg_ps")
                    nc.tensor.matmul(
                        outg_ps[:], adjT_b[:], supp_b[:], start=True, stop=True
                    )
                    nc.vector.tensor_add(outgb[:], outg_ps[:], bb_sb[:])
                    # zero-padded per-t copies so residual matmuls contract
                    # P=18 with partition base 0
                    for t in range(T):
                        nc.any.memset(outg_t[t][:], 0.0)
                        nc.sync.dma_start(
                            outg_t[t][K * t:K * (t + 1), :],
                            outgb[K * t:K * (t + 1), :],
                        )
                    # lhsr[r]: [18, 128] block tile with outg rows 3t:3t+3 in
                    # the column range of each t-span, zeros elsewhere
                    for r in range(NCH):
                        L = lhsr[r]
                        for (t, lo, hi, clo) in _spans(r):
                            nc.any.tensor_copy(
                                L[:, lo:hi], outg_t[t][:, clo:clo + (hi - lo)]
                            )

                # ---------------- pass 2: residual ----------------
                with tc.tile_pool(name="resps", bufs=KNOBS['res_bufs'],
                                  space="PSUM") as rps:
                    order = [8, 6] + list(range(NSTASH)) + [7]
                    for r in order:
                        buf = chunk_buf(r)
                        if r in (6, 7):
                            nc.sync.dma_start(
                                buf[:], x[128 * r:128 * (r + 1), :]
                            )
                        x5 = buf.rearrange(
                            "p (h hh w ww) -> p h hh w ww",
                            h=H0, hh=2, w=W0, ww=2,
                        )
                        for j in range(NR):
                            res = rps.tile([128, RW], F32, tag="res")
                            nc.tensor.matmul(
                                res[:],
                                lhsr[r][:],
                                m56_sb[:, j * RW:(j + 1) * RW],
                                start=True, stop=True,
                            )
                            # ISA caps APs at 3 free dims: split the h-repeat
                            # into two adds (even/odd rows)
                            r4 = res.rearrange("p (h w) -> p h w", w=W0)[
                                :, :, :, None
                            ].to_broadcast((128, 8, W0, 2))
                            for dh in range(2):
                                xs = x5[:, 8 * j:8 * (j + 1), dh]
                                add_eng.tensor_add(xs, xs, r4)
                        store_eng.dma_start(y[128 * r:128 * (r + 1), :], buf[:])
    return nc


def _host_prep(x, gcn_masks, W_emb, W_gcn, b_gcn):
    x = np.asarray(x)
    gcn_masks = np.asarray(gcn_masks)
    wembT = np.asarray(W_emb).T.astype(BF_NP)
    wgcnv = np.ascontiguousarray(np.asarray(W_gcn)).astype(BF_NP)
    bbv = np.ascontiguousarray(
        np.broadcast_to(np.asarray(b_gcn, np.float32)[None, :], (N, C))
    )
    in_maps = []
    for b in range(B):
        xb = np.ascontiguousarray(x[:, b]).reshape(T * C, HW).astype(BF_NP)
        m = gcn_masks[b].reshape(T, K, HW0).astype(BF_NP)
        m56v = np.ascontiguousarray(m.reshape(N, HW0))
        mTpv = np.ascontiguousarray(
            m.reshape(T, K, NJ, JW).transpose(3, 0, 2, 1).reshape(JW, T * NJ * K)
        )
        in_maps.append({
            "x": xb, "m56": m56v, "mTp": mTpv,
            "wembT": wembT, "wgcn": wgcnv, "bb": bbv,
        })
    return in_maps


_NC_CACHE = {}


def kernel(x, gcn_masks, W_emb, W_gcn, b_gcn):
    from concourse.bass_utils import run_bass_kernel_spmd

    in_maps = _host_prep(x, gcn_masks, W_emb, W_gcn, b_gcn)
    if "nc" not in _NC_CACHE:
        _NC_CACHE["nc"] = build_nc(reps=1)
    nc = _NC_CACHE["nc"]
    res = run_bass_kernel_spmd(nc, in_maps, list(range(B)))
    out = np.empty((T, B, C, H, W), np.float32)
    for b in range(B):
        out[:, b] = res.results[b]["y"].astype(np.float32).reshape(T, C, H, W)
    return out


# revision 13
# speedup vs baseline: 2.2790x; 1.0327x over previous
"""Trainium2 Bass kernel for nn_Encoder_90469191122997 (gnn_message_passing).

Data-parallel over batch B=8: core b owns batch b end-to-end.

v2: 16-bit traffic + SBUF residency.  x is sent to the device as bf16
(host cast), y is returned as bf16 (host upcast); the 2e-2 rel-err gate
leaves ~5x margin.  Per core x_b = [T*C, HW] bf16 = 28.9 MB; 7 of the 9
128-row chunks stay resident in SBUF between the pooling pass and the
residual pass, chunk 8 stays in the rotating buffer, and only chunk 7 is
re-read.  HBM traffic/core: 28.9 (read) + 3.2 (re-read) + 28.9 (write)
= 61 MB vs 173 MB for the fp32 two-pass baseline.

Pooling uses a 112-column grid (28 blocks x 112 = 3136, no tail).  Per
chunk, either:
  - DVE path: 2x2 box-sum on DVE (bf16) -> x2, then 28 PE transposes, or
  - PE path (KNOBS['pe_chunks']): 4 accumulating PE matmuls per block
    (x strided slice ^T @ identity) produce the box-summed transpose
    directly in fp32 PSUM, freeing DVE for the pass-2 residual adds.
Transposed tiles are copied PSUM->SBUF on ACT (KNOBS['copy_eng']) and
contracted against the pre-transposed masks -> node features [18, 192].
The 18-node GCN (softmax adjacency, two linears, message passing) runs
on-chip in fp32 PSUM with bf16 operands.  Pass 2 matmuls outg against
the 56x56 masks and DVE adds the 2x-nearest-upsampled residual into the
resident x tiles (broadcast APs), which are then DMA'd out as y.
"""

import numpy as np
import ml_dtypes

import concourse.bass as bass
import concourse.mybir as mybir
import concourse.tile as tile
from concourse.masks import make_identity

T, B, C, H, W = 6, 8, 192, 112, 112
K = 3
H0, W0 = 56, 56
HW = H * W            # 12544
HW0 = H0 * W0         # 3136
N = T * K             # 18
CH = 96               # c half
NJ = 28               # pooling blocks per chunk (112-col grid, no tail)
JW = 112              # pooling block width
NR = 7                # residual hw0 chunks of 448 per row-chunk
RW = 448              # residual chunk width at 56-res (8 rows of 56)
NCH = T * C // 128    # 9 row-chunks of 128 (t,c) rows each
NSTASH = 6            # chunks 0..5 resident in SBUF; 6,7 re-read; 8 in rot

BF = mybir.dt.bfloat16
F32 = mybir.dt.float32
BF_NP = ml_dtypes.bfloat16


def _spans(r):
    """(t, lo, hi, clo): rows [lo,hi) of chunk r belong to t, starting at
    channel clo.  Chunk boundaries hit t-edges only at offsets 0/64."""
    out = []
    for t in range(T):
        lo = max(128 * r, C * t)
        hi = min(128 * r + 128, C * (t + 1))
        if lo < hi:
            out.append((t, lo - 128 * r, hi - 128 * r, lo - C * t))
    return out


_LAST_CHUNK = {t: (C * (t + 1) - 1) // 128 for t in range(T)}

_MAX_WAITS = 1


def _split_multi_waits(nc):
    """This container's walrus rejects >1 sem wait per instruction ("Too many
    sync wait commands").  Move extra waits onto same-engine NoOps inserted
    immediately before the instruction (per-engine program order preserved)."""
    for bb in nc.main_func.blocks:
        insts = list(bb.instructions)
        if not any(
            i.sync_info and i.sync_info.on_wait
            and len(i.sync_info.on_wait) > _MAX_WAITS
            for i in insts
        ):
            continue
        new = []
        for inst in insts:
            si = inst.sync_info
            if si and si.on_wait and len(si.on_wait) > _MAX_WAITS:
                extra = list(si.on_wait[_MAX_WAITS:])
                del si.on_wait[_MAX_WAITS:]
                while extra:
                    chunk, extra = extra[:_MAX_WAITS], extra[_MAX_WAITS:]
                    nop = mybir.InstNoOp(
                        name=nc.get_next_instruction_name(),
                        engine=inst.engine,
                        bass_nofuse=True,
                        sync_info=mybir.SyncInfo(on_wait=chunk, on_update=[]),
                    )
                    nc.register_instruction(nop, overwrite=True)
                    new.append(nop)
            new.append(inst)
        bb.instructions = new


_orig_drain_and_barrier = tile.TileContext._drain_and_barrier


def _patched_drain_and_barrier(self, tick_clock, wait_clock):
    _orig_drain_and_barrier(self, tick_clock, wait_clock)
    _split_multi_waits(self.nc)


tile.TileContext._drain_and_barrier = _patched_drain_and_barrier


KNOBS = dict(
    pe_chunks=(),          # chunks whose box-sum+transpose runs on PE
    copy_eng='scalar',     # engine for PSUM->SBUF transpose-tile copies
    add_eng='vector',      # engine for pass-2 residual adds
    rcopy_eng='vector',    # engine for pass-2 res PSUM->SBUF copies
    store_eng='scalar',    # engine issuing y store DMAs
    x2T_bufs=4, tr_bufs=2, res_bufs=4,
)

if __name__ != "__main__":
    import json as _json
    import os as _os
    _ov = _os.environ.get("KERNEL_KNOBS")
    if _ov:
        KNOBS.update(_json.loads(_ov))


def build_nc(reps: int = 1) -> bass.Bass:
    nc = bass.Bass()
    x = nc.dram_tensor("x", [T * C, HW], BF, kind="ExternalInput")
    m56 = nc.dram_tensor("m56", [N, HW0], BF, kind="ExternalInput")
    mTp = nc.dram_tensor("mTp", [JW, T * NJ * K], BF, kind="ExternalInput")
    wembT = nc.dram_tensor("wembT", [C, C], BF, kind="ExternalInput")
    wgcn = nc.dram_tensor("wgcn", [C, C], BF, kind="ExternalInput")
    bb = nc.dram_tensor("bb", [N, C], F32, kind="ExternalInput")
    y = nc.dram_tensor("y", [T * C, HW], BF, kind="ExternalOutput")

    copy_eng = getattr(nc, KNOBS['copy_eng'])
    add_eng = getattr(nc, KNOBS['add_eng'])
    store_eng = getattr(nc, KNOBS['store_eng'])
    rcopy_eng = getattr(nc, KNOBS['rcopy_eng'])

    with tile.TileContext(nc) as tc:
        with (
            tc.tile_pool(name="persist", bufs=1) as pp,
            tc.tile_pool(name="x2Tpool", bufs=KNOBS['x2T_bufs']) as x2Tpool,
            tc.tile_pool(name="smallsb", bufs=2) as ssb,
        ):
            ident = pp.tile([128, 128], BF)
            make_identity(nc, ident)
            mTp_sb = pp.tile([JW, T * NJ * K], BF)
            nc.sync.dma_start(mTp_sb[:], mTp[:])
            m56_sb = pp.tile([N, HW0], BF)
            nc.sync.dma_start(m56_sb[:], m56[:])
            wemb_h = []
            wgcn_h = []
            for hh in range(2):
                wt = pp.tile([CH, C], BF, tag=f"wemb{hh}")
                nc.sync.dma_start(wt[:], wembT[hh * CH:(hh + 1) * CH, :])
                wemb_h.append(wt)
                gt = pp.tile([CH, C], BF, tag=f"wgcn{hh}")
                nc.sync.dma_start(gt[:], wgcn[hh * CH:(hh + 1) * CH, :])
                wgcn_h.append(gt)
            bb_sb = pp.tile([N, C], F32)
            nc.sync.dma_start(bb_sb[:], bb[:])

            # resident x chunks + rotating buffer + box-sum scratch
            st = [
                pp.tile([128, HW], BF, tag=f"stash{i}", name=f"stash{i}")
                for i in range(NSTASH)
            ]
            rot = pp.tile([128, HW], BF, tag="rot", name="rot")
            x2 = pp.tile([128, HW0], BF, tag="x2", name="x2")

            def chunk_buf(r):
                return st[r] if r < NSTASH else rot

            for rep in range(reps):
                nodeT_h = [
                    pp.tile([CH, N], BF, tag=f"nodeT{hh}", name=f"nodeT{hh}")
                    for hh in range(2)
                ]
                outgb = pp.tile([N, C], BF, tag="outgb", name="outgb")
                outg_t = [
                    pp.tile([N, C], BF, tag=f"outg_t{t}", name=f"outg_t{t}")
                    for t in range(T)
                ]
                lhsr = [
                    pp.tile([N, 128], BF, tag=f"lhsr{r}", name=f"lhsr{r}")
                    for r in range(NCH)
                ]

                # ---------------- pass 1: pooling ----------------
                with (
                    tc.tile_pool(name="trfps", bufs=KNOBS['tr_bufs'],
                                 space="PSUM") as trfps,
                    tc.tile_pool(name="trbps", bufs=KNOBS['tr_bufs'],
                                 space="PSUM") as trbps,
                    tc.tile_pool(name="featps", bufs=3, space="PSUM") as fps,
                    tc.tile_pool(name="ntps", bufs=1, space="PSUM") as ntps,
                ):
                    feat_ps = {}

                    def do_block(r, j, buf, xq):
                        """Produce x2T tile [112, 128] for block j of chunk r
                        and run its pooling matmuls."""
                        if r in KNOBS['pe_chunks']:
                            # 4 accumulating matmuls: out = sum_q plane^T (f32)
                            tr = trfps.tile([JW, 128], F32, tag="trf")
                            for q in range(4):
                                nc.tensor.matmul(
                                    tr[:],
                                    xq[:, q, j * JW:(j + 1) * JW],
                                    ident[:],
                                    start=(q == 0), stop=(q == 3),
                                    skip_group_check=True,
                                )
                        else:
                            tr = trbps.tile([JW, 128], BF, tag="trb")
                            nc.tensor.transpose(
                                tr[:], x2[:, j * JW:(j + 1) * JW], ident[:]
                            )
                        x2T = x2Tpool.tile([JW, 128], BF, tag="x2T")
                        if hasattr(copy_eng, 'tensor_copy'):
                            copy_eng.tensor_copy(x2T[:], tr[:])
                        else:
                            copy_eng.copy(x2T[:], tr[:])
                        for (t, lo, hi, clo) in _spans(r):
                            col = (t * NJ + j) * K
                            nc.tensor.matmul(
                                feat_ps[t][:, clo:clo + (hi - lo)],
                                mTp_sb[:, col:col + K],
                                x2T[:, lo:hi],
                                start=(j == 0), stop=(j == NJ - 1),
                                skip_group_check=True,
                            )

                    for r in range(NCH):
                        buf = chunk_buf(r)
                        nc.sync.dma_start(buf[:], x[128 * r:128 * (r + 1), :])
                        # phase-major layout: buf = [p, (q hw0)] with q the
                        # 2x2 phase (dh, dw); box-sum = 3 step-1 bf16 adds
                        # (DVE 2x packed mode), no strided APs anywhere
                        xq = buf.rearrange("p (q c) -> p q c", q=4)
                        for (t, lo, hi, clo) in _spans(r):
                            if t not in feat_ps:
                                feat_ps[t] = fps.tile(
                                    [K, C], F32, tag="feat_ps",
                                    name=f"featps{t}",
                                )
                        if r in KNOBS['pe_chunks']:
                            for j in range(NJ):
                                do_block(r, j, buf, xq)
                        else:
                            # box-sum in two halves so PE transposes of the
                            # first half overlap DVE summing the second
                            for hf in range(2):
                                sl = slice(hf * (HW0 // 2),
                                           (hf + 1) * (HW0 // 2))
                                out = x2[:, sl]
                                nc.vector.tensor_add(out, xq[:, 0, sl],
                                                     xq[:, 1, sl])
                                nc.vector.tensor_add(out, out, xq[:, 2, sl])
                                nc.vector.tensor_add(out, out, xq[:, 3, sl])
                                for j in range(hf * (NJ // 2),
                                               (hf + 1) * (NJ // 2)):
                                    do_block(r, j, buf, xq)
                        for (t, lo, hi, clo) in _spans(r):
                            if _LAST_CHUNK[t] != r:
                                continue
                            feat_sb = ssb.tile([K, C], BF, tag="feat_sb")
                            nc.scalar.mul(feat_sb[:], feat_ps.pop(t)[:],
                                          1.0 / HW)
                            for hh in range(2):
                                ntr = ntps.tile([CH, K], BF, tag="ntr")
                                nc.tensor.transpose(
                                    ntr[:],
                                    feat_sb[:, hh * CH:(hh + 1) * CH],
                                    ident[:K, :K],
                                )
                                nc.any.tensor_copy(
                                    nodeT_h[hh][:, K * t:K * (t + 1)], ntr[:]
                                )

                # ---------------- GCN on [18, 192] ----------------
                with tc.tile_pool(name="gcnps", bufs=1, space="PSUM") as gps:
                    adjL = gps.tile([N, N], F32, tag="adjL")
                    for hh in range(2):
                        nc.tensor.matmul(
                            adjL[:], nodeT_h[hh][:], nodeT_h[hh][:],
                            start=(hh == 0), stop=(hh == 1),
                        )
                    mx = ssb.tile([N, 1], F32, tag="mx")
                    nc.vector.reduce_max(mx[:], adjL[:], axis=mybir.AxisListType.X)
                    nmx = ssb.tile([N, 1], F32, tag="nmx")
                    nc.vector.tensor_scalar_mul(nmx[:], mx[:], -1.0)
                    e_sb = ssb.tile([N, N], F32, tag="e_sb")
                    nc.scalar.activation(
                        e_sb[:], adjL[:], mybir.ActivationFunctionType.Exp,
                        bias=nmx[:], scale=1.0,
                    )
                    s_ = ssb.tile([N, 1], F32, tag="s_")
                    nc.vector.reduce_sum(s_[:], e_sb[:], axis=mybir.AxisListType.X)
                    r_ = ssb.tile([N, 1], F32, tag="r_")
                    nc.vector.reciprocal(r_[:], s_[:])
                    adj_b = ssb.tile([N, N], BF, tag="adj_b")
                    nc.vector.tensor_scalar_mul(adj_b[:], e_sb[:], r_[:])

                    aaa_ps = gps.tile([N, C], F32, tag="aaa_ps")
                    for hh in range(2):
                        nc.tensor.matmul(
                            aaa_ps[:], nodeT_h[hh][:], wemb_h[hh][:],
                            start=(hh == 0), stop=(hh == 1),
                        )
                    aaa_b = ssb.tile([N, C], BF, tag="aaa_b")
                    nc.scalar.copy(aaa_b[:], aaa_ps[:])
                    aaaT_h = []
                    for hh in range(2):
                        aT_ps = gps.tile([CH, N], BF, tag="aT_ps")
                        nc.tensor.transpose(
                            aT_ps[:], aaa_b[:, hh * CH:(hh + 1) * CH],
                            ident[:N, :N],
                        )
                        aT = ssb.tile([CH, N], BF, tag=f"aaaT{hh}")
                        nc.scalar.copy(aT[:], aT_ps[:])
                        aaaT_h.append(aT)
                    supp_ps = gps.tile([N, C], F32, tag="supp_ps")
                    for hh in range(2):
                        nc.tensor.matmul(
                            supp_ps[:], aaaT_h[hh][:], wgcn_h[hh][:],
                            start=(hh == 0), stop=(hh == 1),
                        )
                    supp_b = ssb.tile([N, C], BF, tag="supp_b")
                    nc.scalar.copy(supp_b[:], supp_ps[:])
                    adjT_ps = gps.tile([N, N], BF, tag="adjT_ps")
                    nc.tensor.transpose(adjT_ps[:], adj_b[:], ident[:N, :N])
                    adjT_b = ssb.tile([N, N], BF, tag="adjT_b")
                    nc.scalar.copy(adjT_b[:], adjT_ps[:])
                    outg_ps = gps.tile([N, C], F32, tag="outg_ps")
                    nc.tensor.matmul(
                        outg_ps[:], adjT_b[:], supp_b[:], start=True, stop=True
                    )
                    nc.vector.tensor_add(outgb[:], outg_ps[:], bb_sb[:])
                    # zero-padded per-t copies so residual matmuls contract
                    # P=18 with partition base 0
                    for t in range(T):
                        nc.any.memset(outg_t[t][:], 0.0)
                        nc.sync.dma_start(
                            outg_t[t][K * t:K * (t + 1), :],
                            outgb[K * t:K * (t + 1), :],
                        )
                    # lhsr[r]: [18, 128] block tile with outg rows 3t:3t+3 in
                    # the column range of each t-span, zeros elsewhere
                    for r in range(NCH):
                        L = lhsr[r]
                        for (t, lo, hi, clo) in _spans(r):
                            nc.any.tensor_copy(
                                L[:, lo:hi], outg_t[t][:, clo:clo + (hi - lo)]
                            )

                # ---------------- pass 2: residual ----------------
                with (
                    tc.tile_pool(name="resps", bufs=KNOBS['res_bufs'],
                                 space="PSUM") as rps,
                    tc.tile_pool(name="ressb", bufs=KNOBS['res_bufs']) as rsb,
                ):
                    order = [8, 6] + list(range(NSTASH)) + [7]
                    for r in order:
                        buf = chunk_buf(r)
                        if r in (6, 7):
                            nc.sync.dma_start(
                                buf[:], x[128 * r:128 * (r + 1), :]
                            )
                        xq = buf.rearrange("p (q c) -> p q c", q=4)
                        for j in range(NR):
                            res = rps.tile([128, RW], F32, tag="res")
                            nc.tensor.matmul(
                                res[:],
                                lhsr[r][:],
                                m56_sb[:, j * RW:(j + 1) * RW],
                                start=True, stop=True,
                            )
                            res_sb = rsb.tile([128, RW], BF, tag="res_sb")
                            rcopy_eng.tensor_copy(res_sb[:], res[:])
                            # nearest-upsample == the same 56-res residual
                            # added to each phase plane: 4 step-1 bf16 adds
                            # (DVE 2x packed mode)
                            sl = slice(j * RW, (j + 1) * RW)
                            for q in range(4):
                                add_eng.tensor_add(xq[:, q, sl], xq[:, q, sl],
                                                   res_sb[:])
                        store_eng.dma_start(y[128 * r:128 * (r + 1), :], buf[:])
    return nc


def _host_prep(x, gcn_masks, W_emb, W_gcn, b_gcn):
    x = np.asarray(x)
    gcn_masks = np.asarray(gcn_masks)
    wembT = np.asarray(W_emb).T.astype(BF_NP)
    wgcnv = np.ascontiguousarray(np.asarray(W_gcn)).astype(BF_NP)
    bbv = np.ascontiguousarray(
        np.broadcast_to(np.asarray(b_gcn, np.float32)[None, :], (N, C))
    )
    in_maps = []
    for b in range(B):
        # phase-major layout: [TC, dh, dw, h0, w0] so the 2x2 box-sum and
        # the nearest-upsample residual add are step-1 ops on device
        xb = np.ascontiguousarray(
            np.asarray(x[:, b]).reshape(T * C, H0, 2, W0, 2)
            .transpose(0, 2, 4, 1, 3).reshape(T * C, HW)
        ).astype(BF_NP)
        m = gcn_masks[b].reshape(T, K, HW0).astype(BF_NP)
        m56v = np.ascontiguousarray(m.reshape(N, HW0))
        mTpv = np.ascontiguousarray(
            m.reshape(T, K, NJ, JW).transpose(3, 0, 2, 1).reshape(JW, T * NJ * K)
        )
        in_maps.append({
            "x": xb, "m56": m56v, "mTp": mTpv,
            "wembT": wembT, "wgcn": wgcnv, "bb": bbv,
        })
    return in_maps


_NC_CACHE = {}


def kernel(x, gcn_masks, W_emb, W_gcn, b_gcn):
    from concourse.bass_utils import run_bass_kernel_spmd

    in_maps = _host_prep(x, gcn_masks, W_emb, W_gcn, b_gcn)
    if "nc" not in _NC_CACHE:
        _NC_CACHE["nc"] = build_nc(reps=1)
    nc = _NC_CACHE["nc"]
    res = run_bass_kernel_spmd(nc, in_maps, list(range(B)))
    out = np.empty((T, B, C, H, W), np.float32)
    for b in range(B):
        yb = res.results[b]["y"].astype(np.float32)
        out[:, b] = (
            yb.reshape(T * C, 2, 2, H0, W0).transpose(0, 3, 1, 4, 2)
            .reshape(T, C, H, W)
        )
    return out
